# revision 48
# baseline (speedup 1.0000x reference)
"""Trainium2 Bass kernel for nn_CLIP topk_masking.

Computes, for full inputs (self-contained; shapes hardcoded):
    probability = image_features @ ima_proto.T          # [B, NP]
    thr_r       = k-th largest of probability row r
    sel[r, j]   = probability[r, j] >= thr_r            # top-k prototype mask
    text_n      = exp(logit_scale) * text_raw / ||text_raw||_row
    logits[r,c] = (image_features @ text_n.T)[r,c] * sel[r, c // 10]

Sharding: data-parallel over the batch axis across 8 NeuronCores
(rows 512/core); prototypes and text features replicated.

Layout notes: all HBM loads use contiguous partition lines (rows 8p..8p+7
on line p -> 16KB DMA packets). The row permutation this induces is undone
by the PE transposes, whose outputs are written through strided SBUF views
so downstream tiles are in natural class/row order. The logit path runs in
bf16 (1 PE cycle/row); the probability path stays fp32 since top-k ranking
decides the mask. Text normalization is folded into the PE transpose by
streaming diag(exp(s)/||t||) instead of the identity.
"""

import os
from contextlib import ExitStack

import numpy as np

import concourse.bass as bass
import concourse.tile as tile
from concourse import bacc, mybir
from concourse.bass_utils import run_bass_kernel_spmd

# Problem shapes (hardcoded per contract).
B, D, NP, NC, CPT = 4096, 512, 1000, 10000, 10
NCORES = 8
RLOC = B // NCORES          # 512 rows per core
RT = RLOC // 128            # 4 row tiles per core
KD = D // 128               # 4 contraction chunks
SC = 1000                   # classes per text superchunk (10 superchunks)
NSC = NC // SC
TPT = 8                     # row tiles per superchunk (1000 = 125*8)
CT = 125                    # partition count for text/proto tiles
GRP = 2                     # superchunks per output store group (2000 cols)
NEG = -1.0e30

F32 = mybir.dt.float32
F32R = mybir.dt.float32r
BF16 = mybir.dt.bfloat16

LAST_RESULTS = None


def _emit(ctx: ExitStack, tc, img, proto, text, out, k: int, inv_s2: float):
    nc = tc.nc
    AF = mybir.ActivationFunctionType
    OP = mybir.AluOpType

    const = ctx.enter_context(tc.tile_pool(name="const", bufs=1))
    persist = ctx.enter_context(tc.tile_pool(name="persist", bufs=1))

    # Identity matrices for PE transposes (f32 for the exact prob path,
    # bf16 as the base for the text diag scaling).
    ones = const.tile([128, 128], F32)
    nc.vector.memset(ones[:], 1.0)
    ident = const.tile([128, 128], F32)
    nc.gpsimd.affine_select(
        ident[:], ones[:], pattern=[[1, 128]], compare_op=OP.is_equal,
        fill=0.0, base=0, channel_multiplier=-1,
    )
    ident_bf = const.tile([128, 128], BF16)
    nc.vector.tensor_copy(ident_bf[:], ident[:])


    # imgT[p, kc, q, t] = img[4q + t, kc*128 + p] (natural row order when
    # the last two dims are flattened). imgT_bf is the bf16 copy for the
    # logit matmul; sel[rt] holds the top-k mask rows 128*rt..128*rt+127.
    imgT = persist.tile([128, KD, 128, RT], F32)
    imgT_bf = persist.tile([128, KD, 128, RT], BF16)
    sels = []

    # Text superchunk loads (2 MB each, 16KB per partition line) on the
    # Sync HWDGE queue; img/proto loads go to the Scalar HWDGE queue so the
    # two streams don't serialize.
    pb_traw = ctx.enter_context(tc.tile_pool(name="pb_traw", bufs=4))
    traw_tiles = {}

    def load_sc(s: int):
        t_ = pb_traw.tile([CT, TPT, D], F32, name=f"traw{s}", tag="traw")
        # Each DMA queue sustains only ~60 GB/s under contention, so the
        # text stream is spread across queues. The first two superchunks
        # ride the SWDGE queue, which is otherwise idle until the first
        # store (~130us): three queues stream in parallel at startup.
        if s < 2:
            eng = nc.gpsimd
        else:
            eng = nc.sync if s % 2 == 0 else nc.scalar
        eng.dma_start(
            t_[:], text[s * SC:(s + 1) * SC].rearrange(
                "(p t) d -> p t d", p=CT))
        traw_tiles[s] = t_

    # Phase-B chain pools (opened early: chains for superchunks 0-1 are
    # emitted during phase A so the PE never starves at the transition).
    pb_bf = ctx.enter_context(tc.tile_pool(name="pb_bf", bufs=2))
    pb_sq = ctx.enter_context(tc.tile_pool(name="pb_sq", bufs=2))
    pb_nrm = ctx.enter_context(tc.tile_pool(name="pb_nrm", bufs=2))

    chains = {}

    def emit_chain(s: int):
        """Per-superchunk normalization chain: row norms + fused
        normalize+bf16-cast (split across Scalar and Vector). The Pool
        engine is kept free: it is an order of magnitude slower than
        DVE/ACT on bulk elementwise work."""
        if s in chains:
            return
        traw = traw_tiles[s]
        nrm = pb_nrm.tile([CT, TPT], F32, tag="nrm")
        for t in range(TPT):
            sq = pb_sq.tile([CT, D], BF16, tag="sq")
            nc.scalar.activation(
                sq[:], traw[:, t], AF.Square, accum_out=nrm[:, t:t + 1])
        # sqrt(||t||^2 * exp(-2*logit_scale)) = ||t|| / s
        nrs = pb_nrm.tile([CT, TPT], F32, tag="nrs")
        nc.scalar.activation(nrs[:], nrm[:], AF.Sqrt, scale=inv_s2)
        rcp = pb_nrm.tile([CT, TPT], F32, tag="rcp")
        nc.vector.reciprocal(rcp[:], nrs[:])       # s / ||t||
        tn_bf = pb_bf.tile([CT, TPT, D], BF16, name=f"tn{s}", tag="tn")
        for t in range(TPT):
            if t % 2 == 0:
                nc.vector.tensor_scalar(
                    tn_bf[:, t], traw[:, t], rcp[:, t:t + 1], None,
                    op0=OP.mult)
            else:
                nc.scalar.mul(tn_bf[:, t], traw[:, t], rcp[:, t:t + 1])
        chains[s] = tn_bf

    # ---------- Phase A: img/proto transpose, probability matmul, top-k ----------
    with (
        tc.tile_pool(name="pa_sb", bufs=1) as pa_sb,
        tc.tile_pool(name="pa_psI", bufs=2, space="PSUM") as pa_psI,
        tc.tile_pool(name="pa_psP", bufs=2, space="PSUM") as pa_psP,
        tc.tile_pool(name="pa_prob_ps", bufs=2, space="PSUM") as pa_prob_ps,
        tc.tile_pool(name="pa_work", bufs=2) as pa_work,
    ):
        # Phase A inputs land in parallel at the head of the two HWDGE
        # queues (img 1MB on sync, proto 2MB on scalar), each queue's text
        # superchunks behind them. Keeps the PE from idling ~50us at the
        # start waiting for phase-A inputs on a congested queue.
        img_sb = pa_sb.tile([128, RT, D], F32)
        nc.sync.dma_start(img_sb[:], img.rearrange("(p t) d -> p t d", p=128))
        proto_sb = pa_sb.tile([CT, TPT, D], F32)
        nc.scalar.dma_start(
            proto_sb[:], proto.rearrange("(p t) d -> p t d", p=CT))
        load_sc(0)
        load_sc(1)
        load_sc(2)

        for t in range(RT):
            pi = pa_psI.tile([128, KD, 128], F32, tag="pi")
            for kc in range(KD):
                nc.tensor.transpose(
                    pi[:, kc], img_sb[:, t, kc * 128:(kc + 1) * 128], ident[:])
            nc.vector.tensor_copy(imgT[:, :, :, t], pi[:])
        nc.scalar.copy(imgT_bf[:], imgT[:])

        # protoT[p, kc, q, t] = proto[8q + t, kc*128 + p] (natural order
        # flattened).
        protoT = pa_sb.tile([128, KD, CT, TPT], F32)
        for t in range(TPT):
            pp = pa_psP.tile([128, KD, CT], F32, tag="pp")
            for kc in range(KD):
                nc.tensor.transpose(
                    pp[:, kc], proto_sb[:, t, kc * 128:(kc + 1) * 128],
                    ident[:CT, :CT])
            nc.vector.tensor_copy(protoT[:, :, :, t], pp[:])

        # Emit the first two text chains BEFORE the prob/topk block: their
        # scalar/vector work then precedes topk in those engines' queues,
        # so the PE gets transposable text as soon as the prob matmuls
        # finish instead of idling ~40us behind the topk emission.
        emit_chain(0)
        emit_chain(1)

        protoT_flat = protoT.rearrange("p kc q t -> p kc (q t)")
        for rt in range(RT):
            ppr = pa_prob_ps.tile([128, 2, 512], F32)
            for h in range(2):
                for kc in range(KD):
                    # fp32 (not bf16): ranking precision decides the mask.
                    nc.tensor.matmul(
                        ppr[:, h, :NP // 2],
                        imgT[:, kc, rt * 32:(rt + 1) * 32, :],
                        protoT_flat[:, kc, h * (NP // 2):(h + 1) * (NP // 2)],
                        start=(kc == 0), stop=(kc == KD - 1),
                    )
            prob = pa_work.tile([128, NP], F32, tag="prob")
            nc.vector.tensor_copy(
                prob[:].rearrange("p (a b) -> p a b", a=2), ppr[:, :, :NP // 2])
            m8a = pa_work.tile([128, 8], F32, tag="m8a")
            nc.vector.max(m8a[:], prob[:])
            if k <= 8:
                thr = m8a[:, k - 1:k]
            else:
                repl = pa_work.tile([128, NP], F32, tag="repl")
                nc.vector.match_replace(repl[:], m8a[:], prob[:], NEG)
                m8b = pa_work.tile([128, 8], F32, tag="m8b")
                nc.vector.max(m8b[:], repl[:])
                thr = m8b[:, k - 9:k - 8]
            sel = persist.tile([128, NP], F32, tag=f"sel{rt}")
            nc.vector.tensor_scalar(sel[:], prob[:], thr, None, op0=OP.is_ge)
            sels.append(sel)

    # ---------- Phase B: text transpose, logit matmul, mask, store ----------
    outv = out.rearrange("(t p) c -> p t c", p=128)
    with (
        tc.tile_pool(name="pb_ttT", bufs=3) as pb_ttT,
        tc.tile_pool(name="pb_psT", bufs=2, space="PSUM") as pb_psT,
        tc.tile_pool(name="pb_psM", bufs=2, space="PSUM") as pb_psM,
        tc.tile_pool(name="pb_stage", bufs=4) as pb_stage,
    ):
        stages = None
        for s in range(NSC):
            if s + 3 < NSC:
                load_sc(s + 3)
            if s + 1 < NSC:
                emit_chain(s + 1)
            tn_bf = chains.pop(s)

            # ttT[p, kc, q, t] = exp(s)/||.|| * text[s*1000 + 8q + t,
            # kc*128 + p]: natural class order when (q, t) is flattened.
            ttT = pb_ttT.tile([128, KD, CT, TPT], BF16, name=f"ttT{s}",
                              tag="ttT")
            for tp in range(TPT // 4):
                # Last dim padded to 128 so each bf16 PSUM slice starts
                # 4-byte aligned (250B offsets are rejected by walrus).
                pt = pb_psT.tile([128, 4, KD, 128], BF16, tag="pt")
                for i in range(4):
                    t = 4 * tp + i
                    for kc in range(KD):
                        nc.tensor.transpose(
                            pt[:, i, kc, :CT],
                            tn_bf[:, t, kc * 128:(kc + 1) * 128],
                            ident_bf[:CT, :CT])
                nc.vector.tensor_copy(
                    ttT[:, :, :, 4 * tp:4 * tp + 4],
                    pt[:, :, :, :CT].rearrange("p i kc q -> p kc q i"))

            g, pos = divmod(s, GRP)
            if pos == 0:
                # Two independent half-stages (rt 0-1 / rt 2-3) per group:
                # each 2MB store DMA releases its 16KB tile separately, so
                # the next group's mask-applies stall half as long.
                stages = [
                    pb_stage.tile([128, 2, GRP * SC], F32,
                                  name=f"stg{g}_{h}", tag="stg")
                    for h in range(2)
                ]
            ttT_flat = ttT.rearrange("p kc q t -> p kc (q t)")
            for rt in range(RT):
                # [128, 2, 512] (not 500): each matmul output must start
                # on a 2KB PSUM bank boundary, and a [.., 1, 0:500] slice
                # at byte 2000 would cross the bank at byte 2048 -> the
                # hardware silently corrupts columns past the boundary.
                pm = pb_psM.tile([128, 2, 512], F32, tag="pm")
                for h in range(2):
                    for kc in range(KD):
                        nc.tensor.matmul(
                            pm[:, h, :NP // 2],
                            imgT_bf[:, kc, rt * 32:(rt + 1) * 32, :],
                            ttT_flat[:, kc, h * (NP // 2):(h + 1) * (NP // 2)],
                            start=(kc == 0), stop=(kc == KD - 1),
                        )
                selb = sels[rt][:, s * (SC // CPT):(s + 1) * (SC // CPT)]
                selb = selb.rearrange("p (h a) -> p h a", h=2)
                selb = selb.broadcast_to([128, 2, SC // CPT // 2, CPT])
                dst = stages[rt // 2][:, rt % 2, pos * SC:(pos + 1) * SC]
                nc.vector.tensor_tensor(
                    dst.rearrange("p (h a b) -> p h a b", h=2, b=CPT),
                    pm[:, :, :NP // 2].rearrange("p h (a b) -> p h a b",
                                                 b=CPT),
                    selb, op=OP.mult)
            if pos == GRP - 1:
                # Two stores of 256 rows x 2000 classes (8KB packets).
                # The last two groups ride the HWDGE queues, whose text
                # loads are finished by then, instead of queueing behind
                # 12MB of earlier stores on the lone SWDGE queue.
                for h in range(2):
                    if g >= 3:
                        seng = nc.sync if h == 0 else nc.scalar
                    else:
                        seng = nc.gpsimd
                    seng.dma_start(
                        outv[:, 2 * h:2 * h + 2,
                             g * GRP * SC:(g + 1) * GRP * SC],
                        stages[h][:])


def _build(k: int, inv_s2: float):
    nc = bacc.Bacc("TRN2", target_bir_lowering=False, debug=False)
    img = nc.dram_tensor("img", [RLOC, D], F32, kind="ExternalInput").ap()
    proto = nc.dram_tensor("proto", [NP, D], F32, kind="ExternalInput").ap()
    text = nc.dram_tensor("text", [NC, D], F32, kind="ExternalInput").ap()
    out = nc.dram_tensor("out", [RLOC, NC], F32, kind="ExternalOutput").ap()
    with tile.TileContext(nc) as tc:
        with ExitStack() as ctx:
            _emit(ctx, tc, img, proto, text, out, k, inv_s2)
    nc.compile()
    return nc


def kernel(image_features, ima_proto, text_features_raw, logit_scale, num_test):
    global LAST_RESULTS
    img = np.ascontiguousarray(np.asarray(image_features, dtype=np.float32))
    proto = np.ascontiguousarray(np.asarray(ima_proto, dtype=np.float32))
    text = np.ascontiguousarray(np.asarray(text_features_raw, dtype=np.float32))
    assert img.shape == (B, D) and proto.shape == (NP, D) and text.shape == (NC, D)
    s = float(np.asarray(logit_scale))
    k = min(int(np.asarray(num_test)), NP)
    assert 1 <= k <= 16, f"kernel supports k in [1, 16], got {k}"
    inv_s2 = float(np.exp(-2.0 * s))

    nc = _build(k, inv_s2)
    in_maps = [
        {"img": img[i * RLOC:(i + 1) * RLOC], "proto": proto, "text": text}
        for i in range(NCORES)
    ]
    trace = bool(int(os.environ.get("BASS_KERNEL_TRACE", "0")))
    res = run_bass_kernel_spmd(nc, in_maps, list(range(NCORES)), trace=trace)
    LAST_RESULTS = res
    return np.concatenate([r["out"] for r in res.results], axis=0)
